# revision 14
# baseline (speedup 1.0000x reference)
"""DenseGeneralAqt inference kernel for Trainium2 (8 NeuronCores).

out = (x @ dequant_int8(qkernel)) * qscale,  x:(2,2048,1024) f32,
qkernel:(1024,4096) int8, qscale:(1,4096) f32 -> out:(2,2048,4096) f32.

Strategy: 2D sharding — 4-way over the flattened token axis (M) x 2-way
over features (N). That minimizes per-core input traffic (2.1 MB x +
2.1 MB w, the HBM-bound startup phase). Input marshalling transposes x
to [D, M] (contraction on SBUF partitions) and casts it to fp16, the
same host pass that shards it. On device each core loads its xT shard
(Sync ring) and its int8 weight half (GPSIMD/SWDGE ring, k-tile 0
first), dequantizes weight k-tiles to fp16 on the vector engine (int8 is
exact in fp16, pipelined ahead of PE consumption), then sweeps m-tile
pairs k-outer across all 8 PSUM banks; the per-channel scale (replicated
across partitions by a deferred DRE-broadcast DMA) is fused into the
PSUM->SBUF drain on the vector engine. The last sweep runs n-outer so
its drains overlap the remaining matmuls. Output stores ride the Scalar
HWDGE ring.
"""

import numpy as np

P = 128
B, S, D, F = 2, 2048, 1024, 4096
N_CORES = 8
MSH, NSH = 4, 2                   # shard grid: 4 m-blocks x 2 n-blocks
M_FULL = B * S                    # 4096 rows
M_CORE = M_FULL // MSH            # 1024 rows per core
N_CORE = F // NSH                 # 2048 cols per core
NT = 512                          # n-tile (one PSUM bank of f32)
WM, WK, WN = M_CORE // P, D // P, N_CORE // NT
XDMA = 8                          # xT load split (1 k-tile per DMA)

_CACHE: dict = {}


def _build():
    import concourse.tile as tile
    from concourse import bacc, mybir

    nc = bacc.Bacc("TRN2", target_bir_lowering=False, debug=False)

    xt_dram = nc.dram_tensor("xt", [D, M_CORE], mybir.dt.float16, kind="ExternalInput")
    w_dram = nc.dram_tensor("w", [D, N_CORE], mybir.dt.int8, kind="ExternalInput")
    s_dram = nc.dram_tensor("s", [1, N_CORE], mybir.dt.float32, kind="ExternalInput")
    o_dram = nc.dram_tensor("o", [M_CORE, N_CORE], mybir.dt.float32, kind="ExternalOutput")

    xt_view = xt_dram[:, :].rearrange("(kt kp) m -> kp kt m", kp=P)  # [128, 8, 1024]

    with tile.TileContext(nc) as tc:
        with (
            tc.tile_pool(name="wi", bufs=1) as wip,
            tc.tile_pool(name="w", bufs=1) as wp,
            tc.tile_pool(name="qs", bufs=1) as qp,
            tc.tile_pool(name="xh", bufs=1) as xhp,
            tc.tile_pool(name="o", bufs=10) as op,
            tc.tile_pool(name="ps", bufs=8, space="PSUM") as pp,
        ):
            # Weight k-tile 0 first (gates the first dequant + matmul).
            w_i8 = [
                wip.tile([P, N_CORE], mybir.dt.int8, name=f"wi{kt}", tag=f"wi{kt}")
                for kt in range(WK)
            ]
            nc.gpsimd.dma_start(w_i8[0][:], w_dram[0:P, :])

            # xT shard [128kp, 8kt, 1024m] fp16 on the Sync ring, one DMA per
            # k-tile. Only chunk 0 races the critical w0 load; the rest are
            # deferred behind the first dequant so w1..w7 arrive sooner.
            xh = xhp.tile([P, WK, M_CORE], mybir.dt.float16, name="xh", tag="xh")
            kper = WK // XDMA
            x_dmas = []
            for i in range(XDMA):
                x_dmas.append(nc.sync.dma_start(
                    xh[:, i * kper:(i + 1) * kper, :],
                    xt_view[:, i * kper:(i + 1) * kper, :],
                ))

            for kt in range(1, WK):
                nc.gpsimd.dma_start(w_i8[kt][:], w_dram[kt * P:(kt + 1) * P, :])

            # Dequant int8 -> fp16 on the vector engine, in k order.
            w_sb = [
                wp.tile([P, N_CORE], mybir.dt.float16, name=f"w{kt}", tag=f"w{kt}")
                for kt in range(WK)
            ]
            cv = [nc.vector.tensor_copy(w_sb[kt][:], w_i8[kt][:]) for kt in range(WK)]
            for i in range(1, XDMA):
                tile.add_dep_helper(x_dmas[i].ins, cv[0].ins, reason="defer x chunk")

            # Scale broadcast (1 MB DRE replication): deferred so its bytes
            # don't starve the critical early loads; lands before first drain.
            qs = qp.tile([P, N_CORE], mybir.dt.float32)
            qs_dma = nc.scalar.dma_start(qs[:], s_dram[0:1, :].to_broadcast((P, N_CORE)))
            tile.add_dep_helper(qs_dma.ins, cv[2].ins, reason="defer qs broadcast")

            def drain(mi, nt, ps_tile):
                ot = op.tile([P, NT], mybir.dt.float32, name=f"o{mi}_{nt}", tag="o")
                nc.vector.tensor_mul(ot[:], ps_tile[:], qs[:, nt * NT:(nt + 1) * NT])
                nc.scalar.dma_start(
                    o_dram[mi * P:(mi + 1) * P, nt * NT:(nt + 1) * NT], ot[:]
                )

            def mm(ps_tile, kt, mi, nt, first, last):
                nc.tensor.matmul(
                    ps_tile[:],
                    xh[:, kt, mi * P:(mi + 1) * P],
                    w_sb[kt][:, nt * NT:(nt + 1) * NT],
                    start=first,
                    stop=last,
                )

            # m-tile pairs x 4 n-tiles = 8 PSUM banks per k-outer sweep.
            pairs = [(2 * i, 2 * i + 1) for i in range(WM // 2)]
            for pi, pair in enumerate(pairs):
                combos = [(mi, nt) for mi in pair for nt in range(WN)]
                if pi < len(pairs) - 1:
                    # k-outer: consume each weight k-tile across all 8 banks
                    # as soon as it is dequantized.
                    ps = {
                        c: pp.tile([P, NT], mybir.dt.float32, name=f"ps{pi}_{c[0]}_{c[1]}", tag="ps")
                        for c in combos
                    }
                    for kt in range(WK):
                        for c in combos:
                            mm(ps[c], kt, c[0], c[1], kt == 0, kt == WK - 1)
                    for c in combos:
                        drain(c[0], c[1], ps[c])
                else:
                    # Last sweep: n-outer so each bank's reduction finishes
                    # early and the tail drains overlap the remaining matmuls.
                    for c in combos:
                        ps_t = pp.tile([P, NT], mybir.dt.float32, name=f"ps{pi}_{c[0]}_{c[1]}", tag="ps")
                        for kt in range(WK):
                            mm(ps_t, kt, c[0], c[1], kt == 0, kt == WK - 1)
                        drain(c[0], c[1], ps_t)

    nc.compile()
    return nc


def _get_nc():
    if "nc" not in _CACHE:
        _CACHE["nc"] = _build()
    return _CACHE["nc"]


def _run(x, qkernel, qscale, trace=False):
    from concourse.bass_utils import run_bass_kernel_spmd

    x = np.asarray(x, dtype=np.float32).reshape(M_FULL, D)
    xt = np.ascontiguousarray(x.T).astype(np.float16)    # [D, M_FULL]
    w = np.asarray(qkernel)
    if w.dtype != np.int8:
        w = w.astype(np.int8)
    s = np.asarray(qscale, dtype=np.float32).reshape(1, F)

    in_maps = []
    for c in range(N_CORES):
        mb, nb = c % MSH, c // MSH
        in_maps.append({
            "xt": np.ascontiguousarray(xt[:, mb * M_CORE:(mb + 1) * M_CORE]),
            "w": np.ascontiguousarray(w[:, nb * N_CORE:(nb + 1) * N_CORE]),
            "s": np.ascontiguousarray(s[:, nb * N_CORE:(nb + 1) * N_CORE]),
        })
    res = run_bass_kernel_spmd(
        _get_nc(), in_maps, core_ids=list(range(N_CORES)), trace=trace
    )
    out = np.empty((M_FULL, F), dtype=np.float32)
    for c in range(N_CORES):
        mb, nb = c % MSH, c // MSH
        out[mb * M_CORE:(mb + 1) * M_CORE, nb * N_CORE:(nb + 1) * N_CORE] = res.results[c]["o"]
    return out.reshape(B, S, F), res


def kernel(x, qkernel, qscale):
    out, _ = _run(x, qkernel, qscale, trace=False)
    return out


def kernel_traced(x, qkernel, qscale):
    out, res = _run(x, qkernel, qscale, trace=True)
    return out, res


# revision 16
# speedup vs baseline: 1.0260x; 1.0260x over previous
"""DenseGeneralAqt inference kernel for Trainium2 (8 NeuronCores).

out = (x @ dequant_int8(qkernel)) * qscale,  x:(2,2048,1024) f32,
qkernel:(1024,4096) int8, qscale:(1,4096) f32 -> out:(2,2048,4096) f32.

Strategy: 2D sharding — 4-way over the flattened token axis (M) x 2-way
over features (N). That minimizes per-core input traffic (2.1 MB x +
2.1 MB w, the HBM-bound startup phase). Input marshalling transposes x
to [D, M] (contraction on SBUF partitions) and casts it to fp16, the
same host pass that shards it. On device each core loads its xT shard
(Sync ring) and its int8 weight half (GPSIMD/SWDGE ring, k-tile 0
first), dequantizes weight k-tiles to fp16 on the vector engine (int8 is
exact in fp16, pipelined ahead of PE consumption), then sweeps m-tile
pairs k-outer across all 8 PSUM banks; the per-channel scale (replicated
across partitions by a deferred DRE-broadcast DMA) is fused into the
PSUM->SBUF drain on the vector engine. The last sweep runs n-outer so
its drains overlap the remaining matmuls. Output stores ride the Scalar
HWDGE ring.
"""

import numpy as np

P = 128
B, S, D, F = 2, 2048, 1024, 4096
N_CORES = 8
MSH, NSH = 4, 2                   # shard grid: 4 m-blocks x 2 n-blocks
M_FULL = B * S                    # 4096 rows
M_CORE = M_FULL // MSH            # 1024 rows per core
N_CORE = F // NSH                 # 2048 cols per core
NT = 512                          # n-tile (one PSUM bank of f32)
WM, WK, WN = M_CORE // P, D // P, N_CORE // NT
XDMA = 8                          # xT load split (1 k-tile per DMA)

_CACHE: dict = {}


def _build():
    import concourse.tile as tile
    from concourse import bacc, mybir

    nc = bacc.Bacc("TRN2", target_bir_lowering=False, debug=False)

    xt_dram = nc.dram_tensor("xt", [D, M_CORE], mybir.dt.float16, kind="ExternalInput")
    w_dram = nc.dram_tensor("w", [D, N_CORE], mybir.dt.int8, kind="ExternalInput")
    s_dram = nc.dram_tensor("s", [1, N_CORE], mybir.dt.float32, kind="ExternalInput")
    o_dram = nc.dram_tensor("o", [M_CORE, N_CORE], mybir.dt.float32, kind="ExternalOutput")

    xt_view = xt_dram[:, :].rearrange("(kt kp) m -> kp kt m", kp=P)  # [128, 8, 1024]

    with tile.TileContext(nc) as tc:
        with (
            tc.tile_pool(name="wi", bufs=1) as wip,
            tc.tile_pool(name="w", bufs=1) as wp,
            tc.tile_pool(name="qs", bufs=1) as qp,
            tc.tile_pool(name="xh", bufs=1) as xhp,
            tc.tile_pool(name="o", bufs=10) as op,
            tc.tile_pool(name="ps", bufs=8, space="PSUM") as pp,
        ):
            # Weight k-tile 0 first (gates the first dequant + matmul).
            w_i8 = [
                wip.tile([P, N_CORE], mybir.dt.int8, name=f"wi{kt}", tag=f"wi{kt}")
                for kt in range(WK)
            ]
            w0_dma = nc.gpsimd.dma_start(w_i8[0][:], w_dram[0:P, :])

            # xT shard [128kp, 8kt, 1024m] fp16 on the Sync ring, one DMA per
            # k-tile. Only chunk 0 races the critical w0 load; the rest are
            # deferred behind the first dequant so w1..w7 arrive sooner.
            xh = xhp.tile([P, WK, M_CORE], mybir.dt.float16, name="xh", tag="xh")
            kper = WK // XDMA
            x_dmas = []
            for i in range(XDMA):
                x_dmas.append(nc.sync.dma_start(
                    xh[:, i * kper:(i + 1) * kper, :],
                    xt_view[:, i * kper:(i + 1) * kper, :],
                ))

            for kt in range(1, WK):
                nc.gpsimd.dma_start(w_i8[kt][:], w_dram[kt * P:(kt + 1) * P, :])

            # Dequant int8 -> fp16 on the vector engine, in k order.
            w_sb = [
                wp.tile([P, N_CORE], mybir.dt.float16, name=f"w{kt}", tag=f"w{kt}")
                for kt in range(WK)
            ]
            cv = [nc.vector.tensor_copy(w_sb[kt][:], w_i8[kt][:]) for kt in range(WK)]
            for i in range(2, XDMA):
                tile.add_dep_helper(x_dmas[i].ins, w0_dma.ins, reason="defer x chunk")

            # Scale broadcast (1 MB DRE replication): deferred so its bytes
            # don't starve the critical early loads; lands before first drain.
            qs = qp.tile([P, N_CORE], mybir.dt.float32)
            qs_dma = nc.scalar.dma_start(qs[:], s_dram[0:1, :].to_broadcast((P, N_CORE)))
            tile.add_dep_helper(qs_dma.ins, cv[2].ins, reason="defer qs broadcast")

            def drain(mi, nt, ps_tile):
                ot = op.tile([P, NT], mybir.dt.float32, name=f"o{mi}_{nt}", tag="o")
                nc.vector.tensor_mul(ot[:], ps_tile[:], qs[:, nt * NT:(nt + 1) * NT])
                nc.scalar.dma_start(
                    o_dram[mi * P:(mi + 1) * P, nt * NT:(nt + 1) * NT], ot[:]
                )

            def mm(ps_tile, kt, mi, nt, first, last):
                nc.tensor.matmul(
                    ps_tile[:],
                    xh[:, kt, mi * P:(mi + 1) * P],
                    w_sb[kt][:, nt * NT:(nt + 1) * NT],
                    start=first,
                    stop=last,
                )

            # m-tile pairs x 4 n-tiles = 8 PSUM banks per k-outer sweep.
            pairs = [(2 * i, 2 * i + 1) for i in range(WM // 2)]
            for pi, pair in enumerate(pairs):
                combos = [(mi, nt) for mi in pair for nt in range(WN)]
                if pi < len(pairs) - 1:
                    # k-outer: consume each weight k-tile across all 8 banks
                    # as soon as it is dequantized.
                    ps = {
                        c: pp.tile([P, NT], mybir.dt.float32, name=f"ps{pi}_{c[0]}_{c[1]}", tag="ps")
                        for c in combos
                    }
                    for kt in range(WK):
                        for c in combos:
                            mm(ps[c], kt, c[0], c[1], kt == 0, kt == WK - 1)
                    for c in combos:
                        drain(c[0], c[1], ps[c])
                else:
                    # Last sweep: n-outer so each bank's reduction finishes
                    # early and the tail drains overlap the remaining matmuls.
                    for c in combos:
                        ps_t = pp.tile([P, NT], mybir.dt.float32, name=f"ps{pi}_{c[0]}_{c[1]}", tag="ps")
                        for kt in range(WK):
                            mm(ps_t, kt, c[0], c[1], kt == 0, kt == WK - 1)
                        drain(c[0], c[1], ps_t)

    nc.compile()
    return nc


def _get_nc():
    if "nc" not in _CACHE:
        _CACHE["nc"] = _build()
    return _CACHE["nc"]


def _run(x, qkernel, qscale, trace=False):
    from concourse.bass_utils import run_bass_kernel_spmd

    x = np.asarray(x, dtype=np.float32).reshape(M_FULL, D)
    xt = np.ascontiguousarray(x.T).astype(np.float16)    # [D, M_FULL]
    w = np.asarray(qkernel)
    if w.dtype != np.int8:
        w = w.astype(np.int8)
    s = np.asarray(qscale, dtype=np.float32).reshape(1, F)

    in_maps = []
    for c in range(N_CORES):
        mb, nb = c % MSH, c // MSH
        in_maps.append({
            "xt": np.ascontiguousarray(xt[:, mb * M_CORE:(mb + 1) * M_CORE]),
            "w": np.ascontiguousarray(w[:, nb * N_CORE:(nb + 1) * N_CORE]),
            "s": np.ascontiguousarray(s[:, nb * N_CORE:(nb + 1) * N_CORE]),
        })
    res = run_bass_kernel_spmd(
        _get_nc(), in_maps, core_ids=list(range(N_CORES)), trace=trace
    )
    out = np.empty((M_FULL, F), dtype=np.float32)
    for c in range(N_CORES):
        mb, nb = c % MSH, c // MSH
        out[mb * M_CORE:(mb + 1) * M_CORE, nb * N_CORE:(nb + 1) * N_CORE] = res.results[c]["o"]
    return out.reshape(B, S, F), res


def kernel(x, qkernel, qscale):
    out, _ = _run(x, qkernel, qscale, trace=False)
    return out


def kernel_traced(x, qkernel, qscale):
    out, res = _run(x, qkernel, qscale, trace=True)
    return out, res


# revision 17
# speedup vs baseline: 1.0444x; 1.0179x over previous
"""DenseGeneralAqt inference kernel for Trainium2 (8 NeuronCores).

out = (x @ dequant_int8(qkernel)) * qscale,  x:(2,2048,1024) f32,
qkernel:(1024,4096) int8, qscale:(1,4096) f32 -> out:(2,2048,4096) f32.

Strategy: 2D sharding — 4-way over the flattened token axis (M) x 2-way
over features (N). That minimizes per-core input traffic (2.1 MB x +
2.1 MB w, the HBM-bound startup phase). Input marshalling transposes x
to [D, M] (contraction on SBUF partitions) and casts it to fp16, the
same host pass that shards it. On device each core loads its xT shard
(Sync ring) and its int8 weight half (GPSIMD/SWDGE ring, k-tile 0
first), dequantizes weight k-tiles to fp16 on the vector engine (int8 is
exact in fp16, pipelined ahead of PE consumption), then sweeps m-tile
pairs k-outer across all 8 PSUM banks; the per-channel scale (replicated
across partitions by a deferred DRE-broadcast DMA) is fused into the
PSUM->SBUF drain on the vector engine. The last sweep runs n-outer so
its drains overlap the remaining matmuls. Output stores ride the Scalar
HWDGE ring.
"""

import numpy as np

P = 128
B, S, D, F = 2, 2048, 1024, 4096
N_CORES = 8
MSH, NSH = 4, 2                   # shard grid: 4 m-blocks x 2 n-blocks
M_FULL = B * S                    # 4096 rows
M_CORE = M_FULL // MSH            # 1024 rows per core
N_CORE = F // NSH                 # 2048 cols per core
NT = 512                          # n-tile (one PSUM bank of f32)
WM, WK, WN = M_CORE // P, D // P, N_CORE // NT
XDMA = 4                          # xT load split (2 k-tiles per DMA)

_CACHE: dict = {}


def _build():
    import concourse.tile as tile
    from concourse import bacc, mybir

    nc = bacc.Bacc("TRN2", target_bir_lowering=False, debug=False)

    xt_dram = nc.dram_tensor("xt", [D, M_CORE], mybir.dt.float16, kind="ExternalInput")
    w_dram = nc.dram_tensor("w", [D, N_CORE], mybir.dt.int8, kind="ExternalInput")
    s_dram = nc.dram_tensor("s", [1, N_CORE], mybir.dt.float32, kind="ExternalInput")
    o_dram = nc.dram_tensor("o", [M_CORE, N_CORE], mybir.dt.float32, kind="ExternalOutput")

    xt_view = xt_dram[:, :].rearrange("(kt kp) m -> kp kt m", kp=P)  # [128, 8, 1024]

    with tile.TileContext(nc) as tc:
        with (
            tc.tile_pool(name="wi", bufs=1) as wip,
            tc.tile_pool(name="w", bufs=1) as wp,
            tc.tile_pool(name="qs", bufs=1) as qp,
            tc.tile_pool(name="xh", bufs=1) as xhp,
            tc.tile_pool(name="o", bufs=10) as op,
            tc.tile_pool(name="ps", bufs=8, space="PSUM") as pp,
        ):
            # Weight k-tile 0 first (gates the first dequant + matmul).
            w_i8 = [
                wip.tile([P, N_CORE], mybir.dt.int8, name=f"wi{kt}", tag=f"wi{kt}")
                for kt in range(WK)
            ]
            nc.gpsimd.dma_start(w_i8[0][:], w_dram[0:P, :])

            # xT shard [128kp, 8kt, 1024m] fp16 on the Sync ring.
            xh = xhp.tile([P, WK, M_CORE], mybir.dt.float16, name="xh", tag="xh")
            kper = WK // XDMA
            for i in range(XDMA):
                nc.sync.dma_start(
                    xh[:, i * kper:(i + 1) * kper, :],
                    xt_view[:, i * kper:(i + 1) * kper, :],
                )

            for kt in range(1, WK):
                nc.gpsimd.dma_start(w_i8[kt][:], w_dram[kt * P:(kt + 1) * P, :])

            # Dequant int8 -> fp16 on the vector engine, in k order.
            w_sb = [
                wp.tile([P, N_CORE], mybir.dt.float16, name=f"w{kt}", tag=f"w{kt}")
                for kt in range(WK)
            ]
            cv = [nc.vector.tensor_copy(w_sb[kt][:], w_i8[kt][:]) for kt in range(WK)]

            # Scale broadcast (1 MB DRE replication): deferred so its bytes
            # don't starve the critical early loads; lands before first drain.
            qs = qp.tile([P, N_CORE], mybir.dt.float32)
            qs_dma = nc.scalar.dma_start(qs[:], s_dram[0:1, :].to_broadcast((P, N_CORE)))
            tile.add_dep_helper(qs_dma.ins, cv[2].ins, reason="defer qs broadcast")

            def drain(mi, nt, ps_tile):
                ot = op.tile([P, NT], mybir.dt.float32, name=f"o{mi}_{nt}", tag="o")
                nc.vector.tensor_mul(ot[:], ps_tile[:], qs[:, nt * NT:(nt + 1) * NT])
                nc.scalar.dma_start(
                    o_dram[mi * P:(mi + 1) * P, nt * NT:(nt + 1) * NT], ot[:]
                )

            def mm(ps_tile, kt, mi, nt, first, last):
                nc.tensor.matmul(
                    ps_tile[:],
                    xh[:, kt, mi * P:(mi + 1) * P],
                    w_sb[kt][:, nt * NT:(nt + 1) * NT],
                    start=first,
                    stop=last,
                )

            # m-tile pairs x 4 n-tiles = 8 PSUM banks per k-outer sweep.
            pairs = [(2 * i, 2 * i + 1) for i in range(WM // 2)]
            for pi, pair in enumerate(pairs):
                combos = [(mi, nt) for mi in pair for nt in range(WN)]
                if pi < len(pairs) - 1:
                    # k-outer: consume each weight k-tile across all 8 banks
                    # as soon as it is dequantized.
                    ps = {
                        c: pp.tile([P, NT], mybir.dt.float32, name=f"ps{pi}_{c[0]}_{c[1]}", tag="ps")
                        for c in combos
                    }
                    for kt in range(WK):
                        for c in combos:
                            mm(ps[c], kt, c[0], c[1], kt == 0, kt == WK - 1)
                    for c in combos:
                        drain(c[0], c[1], ps[c])
                else:
                    # Last sweep: n-outer so each bank's reduction finishes
                    # early and the tail drains overlap the remaining matmuls.
                    for c in combos:
                        ps_t = pp.tile([P, NT], mybir.dt.float32, name=f"ps{pi}_{c[0]}_{c[1]}", tag="ps")
                        for kt in range(WK):
                            mm(ps_t, kt, c[0], c[1], kt == 0, kt == WK - 1)
                        drain(c[0], c[1], ps_t)

    nc.compile()
    return nc


def _get_nc():
    if "nc" not in _CACHE:
        _CACHE["nc"] = _build()
    return _CACHE["nc"]


def _run(x, qkernel, qscale, trace=False):
    from concourse.bass_utils import run_bass_kernel_spmd

    x = np.asarray(x, dtype=np.float32).reshape(M_FULL, D)
    xt = np.ascontiguousarray(x.T).astype(np.float16)    # [D, M_FULL]
    w = np.asarray(qkernel)
    if w.dtype != np.int8:
        w = w.astype(np.int8)
    s = np.asarray(qscale, dtype=np.float32).reshape(1, F)

    in_maps = []
    for c in range(N_CORES):
        mb, nb = c % MSH, c // MSH
        in_maps.append({
            "xt": np.ascontiguousarray(xt[:, mb * M_CORE:(mb + 1) * M_CORE]),
            "w": np.ascontiguousarray(w[:, nb * N_CORE:(nb + 1) * N_CORE]),
            "s": np.ascontiguousarray(s[:, nb * N_CORE:(nb + 1) * N_CORE]),
        })
    res = run_bass_kernel_spmd(
        _get_nc(), in_maps, core_ids=list(range(N_CORES)), trace=trace
    )
    out = np.empty((M_FULL, F), dtype=np.float32)
    for c in range(N_CORES):
        mb, nb = c % MSH, c // MSH
        out[mb * M_CORE:(mb + 1) * M_CORE, nb * N_CORE:(nb + 1) * N_CORE] = res.results[c]["o"]
    return out.reshape(B, S, F), res


def kernel(x, qkernel, qscale):
    out, _ = _run(x, qkernel, qscale, trace=False)
    return out


def kernel_traced(x, qkernel, qscale):
    out, res = _run(x, qkernel, qscale, trace=True)
    return out, res


# revision 21
# speedup vs baseline: 1.0487x; 1.0041x over previous
"""DenseGeneralAqt inference kernel for Trainium2 (8 NeuronCores).

out = (x @ dequant_int8(qkernel)) * qscale,  x:(2,2048,1024) f32,
qkernel:(1024,4096) int8, qscale:(1,4096) f32 -> out:(2,2048,4096) f32.

Strategy: 2D sharding — 4-way over the flattened token axis (M) x 2-way
over features (N). That minimizes per-core input traffic (2.1 MB x +
2.1 MB w, the HBM-bound startup phase). Input marshalling transposes x
to [D, M] (contraction on SBUF partitions) and casts it to fp16, the
same host pass that shards it. On device each core loads its xT shard
(Sync ring) and its int8 weight half (GPSIMD/SWDGE ring, k-tile 0
first), dequantizes weight k-tiles to fp16 on the vector engine (int8 is
exact in fp16, pipelined ahead of PE consumption), then sweeps m-tile
pairs k-outer across all 8 PSUM banks; the per-channel scale (replicated
across partitions by a deferred DRE-broadcast DMA) is fused into the
PSUM->SBUF drain on the vector engine. The last sweep runs n-outer so
its drains overlap the remaining matmuls. Output stores ride the Scalar
HWDGE ring.
"""

import numpy as np

P = 128
B, S, D, F = 2, 2048, 1024, 4096
N_CORES = 8
MSH, NSH = 4, 2                   # shard grid: 4 m-blocks x 2 n-blocks
M_FULL = B * S                    # 4096 rows
M_CORE = M_FULL // MSH            # 1024 rows per core
N_CORE = F // NSH                 # 2048 cols per core
NT = 512                          # n-tile (one PSUM bank of f32)
WM, WK, WN = M_CORE // P, D // P, N_CORE // NT
XDMA = 4                          # xT load split (2 k-tiles per DMA)

_CACHE: dict = {}


def _build():
    import concourse.tile as tile
    from concourse import bacc, mybir

    nc = bacc.Bacc("TRN2", target_bir_lowering=False, debug=False)

    xt_dram = nc.dram_tensor("xt", [D, M_CORE], mybir.dt.float16, kind="ExternalInput")
    w_dram = nc.dram_tensor("w", [D, N_CORE], mybir.dt.int8, kind="ExternalInput")
    s_dram = nc.dram_tensor("s", [1, N_CORE], mybir.dt.float32, kind="ExternalInput")
    o_dram = nc.dram_tensor("o", [M_CORE, N_CORE], mybir.dt.float32, kind="ExternalOutput")

    xt_view = xt_dram[:, :].rearrange("(kt kp) m -> kp kt m", kp=P)  # [128, 8, 1024]

    with tile.TileContext(nc) as tc:
        with (
            tc.tile_pool(name="wi", bufs=1) as wip,
            tc.tile_pool(name="w", bufs=1) as wp,
            tc.tile_pool(name="qs", bufs=1) as qp,
            tc.tile_pool(name="xh", bufs=1) as xhp,
            tc.tile_pool(name="o", bufs=10) as op,
            tc.tile_pool(name="ps", bufs=8, space="PSUM") as pp,
        ):
            # Weight k-tile 0 first (gates the first dequant + matmul); its
            # first half alone gates the first matmul, so load it separately.
            w_i8 = [
                wip.tile([P, N_CORE], mybir.dt.int8, name=f"wi{kt}", tag=f"wi{kt}")
                for kt in range(WK)
            ]
            wh = N_CORE // 2
            nc.gpsimd.dma_start(w_i8[0][:, 0:wh], w_dram[0:P, 0:wh])
            nc.gpsimd.dma_start(w_i8[0][:, wh:N_CORE], w_dram[0:P, wh:N_CORE])

            # PE warm-up: ~3.6us of dummy matmuls on zeros while loads are in
            # flight, so the HAM clock-gate is released (1.2 -> 2.4 GHz)
            # before the first real matmul issues.
            warm = wp.tile([P, NT], mybir.dt.float16, name="warm", tag="warm")
            nc.vector.memset(warm[:], 0)
            warm_ps = pp.tile([P, NT], mybir.dt.float32, name="warm_ps", tag="ps")
            for _ in range(36):
                nc.tensor.matmul(warm_ps[:, 0:P], warm[:, 0:P], warm[:, 0:P])

            # xT shard [128kp, 8kt, 1024m] fp16 on the Sync ring.
            xh = xhp.tile([P, WK, M_CORE], mybir.dt.float16, name="xh", tag="xh")
            kper = WK // XDMA
            for i in range(XDMA):
                nc.sync.dma_start(
                    xh[:, i * kper:(i + 1) * kper, :],
                    xt_view[:, i * kper:(i + 1) * kper, :],
                )

            for kt in range(1, WK):
                nc.gpsimd.dma_start(w_i8[kt][:], w_dram[kt * P:(kt + 1) * P, :])

            # Dequant int8 -> fp16 on the vector engine, in k order; k-tile 0
            # in halves so the first matmuls' columns are ready earliest.
            w_sb = [
                wp.tile([P, N_CORE], mybir.dt.float16, name=f"w{kt}", tag=f"w{kt}")
                for kt in range(WK)
            ]
            nc.vector.tensor_copy(w_sb[0][:, 0:wh], w_i8[0][:, 0:wh])
            nc.vector.tensor_copy(w_sb[0][:, wh:N_CORE], w_i8[0][:, wh:N_CORE])
            cv = [nc.vector.tensor_copy(w_sb[kt][:], w_i8[kt][:]) for kt in range(1, WK)]

            # Scale broadcast (1 MB DRE replication): deferred so its bytes
            # don't starve the critical early loads; lands before first drain.
            qs = qp.tile([P, N_CORE], mybir.dt.float32)
            qs_dma = nc.scalar.dma_start(qs[:], s_dram[0:1, :].to_broadcast((P, N_CORE)))
            tile.add_dep_helper(qs_dma.ins, cv[1].ins, reason="defer qs broadcast")

            def drain(mi, nt, ps_tile):
                ot = op.tile([P, NT], mybir.dt.float32, name=f"o{mi}_{nt}", tag="o")
                nc.vector.tensor_mul(ot[:], ps_tile[:], qs[:, nt * NT:(nt + 1) * NT])
                nc.scalar.dma_start(
                    o_dram[mi * P:(mi + 1) * P, nt * NT:(nt + 1) * NT], ot[:]
                )

            def mm(ps_tile, kt, mi, nt, first, last):
                nc.tensor.matmul(
                    ps_tile[:],
                    xh[:, kt, mi * P:(mi + 1) * P],
                    w_sb[kt][:, nt * NT:(nt + 1) * NT],
                    start=first,
                    stop=last,
                )

            # m-tile pairs x 4 n-tiles = 8 PSUM banks per k-outer sweep.
            pairs = [(2 * i, 2 * i + 1) for i in range(WM // 2)]
            for pi, pair in enumerate(pairs):
                combos = [(mi, nt) for mi in pair for nt in range(WN)]
                if pi < len(pairs) - 1:
                    # k-outer: consume each weight k-tile across all 8 banks
                    # as soon as it is dequantized.
                    ps = {
                        c: pp.tile([P, NT], mybir.dt.float32, name=f"ps{pi}_{c[0]}_{c[1]}", tag="ps")
                        for c in combos
                    }
                    for kt in range(WK):
                        for c in combos:
                            mm(ps[c], kt, c[0], c[1], kt == 0, kt == WK - 1)
                    for c in combos:
                        drain(c[0], c[1], ps[c])
                else:
                    # Last sweep: n-outer so each bank's reduction finishes
                    # early and the tail drains overlap the remaining matmuls.
                    for c in combos:
                        ps_t = pp.tile([P, NT], mybir.dt.float32, name=f"ps{pi}_{c[0]}_{c[1]}", tag="ps")
                        for kt in range(WK):
                            mm(ps_t, kt, c[0], c[1], kt == 0, kt == WK - 1)
                        drain(c[0], c[1], ps_t)

    nc.compile()
    return nc


def _get_nc():
    if "nc" not in _CACHE:
        _CACHE["nc"] = _build()
    return _CACHE["nc"]


def _run(x, qkernel, qscale, trace=False):
    from concourse.bass_utils import run_bass_kernel_spmd

    x = np.asarray(x, dtype=np.float32).reshape(M_FULL, D)
    xt = np.ascontiguousarray(x.T).astype(np.float16)    # [D, M_FULL]
    w = np.asarray(qkernel)
    if w.dtype != np.int8:
        w = w.astype(np.int8)
    s = np.asarray(qscale, dtype=np.float32).reshape(1, F)

    in_maps = []
    for c in range(N_CORES):
        mb, nb = c % MSH, c // MSH
        in_maps.append({
            "xt": np.ascontiguousarray(xt[:, mb * M_CORE:(mb + 1) * M_CORE]),
            "w": np.ascontiguousarray(w[:, nb * N_CORE:(nb + 1) * N_CORE]),
            "s": np.ascontiguousarray(s[:, nb * N_CORE:(nb + 1) * N_CORE]),
        })
    res = run_bass_kernel_spmd(
        _get_nc(), in_maps, core_ids=list(range(N_CORES)), trace=trace
    )
    out = np.empty((M_FULL, F), dtype=np.float32)
    for c in range(N_CORES):
        mb, nb = c % MSH, c // MSH
        out[mb * M_CORE:(mb + 1) * M_CORE, nb * N_CORE:(nb + 1) * N_CORE] = res.results[c]["o"]
    return out.reshape(B, S, F), res


def kernel(x, qkernel, qscale):
    out, _ = _run(x, qkernel, qscale, trace=False)
    return out


def kernel_traced(x, qkernel, qscale):
    out, res = _run(x, qkernel, qscale, trace=True)
    return out, res


# revision 26
# speedup vs baseline: 1.0498x; 1.0011x over previous
"""DenseGeneralAqt inference kernel for Trainium2 (8 NeuronCores).

out = (x @ dequant_int8(qkernel)) * qscale,  x:(2,2048,1024) f32,
qkernel:(1024,4096) int8, qscale:(1,4096) f32 -> out:(2,2048,4096) f32.

Strategy: 2D sharding — 4-way over the flattened token axis (M) x 2-way
over features (N). That minimizes per-core input traffic (2.1 MB x +
2.1 MB w, the HBM-bound startup phase). Input marshalling transposes x
to [D, M] (contraction on SBUF partitions) and casts it to fp16, the
same host pass that shards it. On device each core loads its xT shard
(Sync ring) and its int8 weight half (GPSIMD/SWDGE ring, k-tile 0
first), dequantizes weight k-tiles to fp16 on the vector engine (int8 is
exact in fp16, pipelined ahead of PE consumption), then sweeps m-tile
pairs k-outer across all 8 PSUM banks; the per-channel scale (replicated
across partitions by a deferred DRE-broadcast DMA) is fused into the
PSUM->SBUF drain on the vector engine. The last sweep runs n-outer so
its drains overlap the remaining matmuls. Output stores ride the Scalar
HWDGE ring.
"""

import numpy as np

P = 128
B, S, D, F = 2, 2048, 1024, 4096
N_CORES = 8
MSH, NSH = 4, 2                   # shard grid: 4 m-blocks x 2 n-blocks
M_FULL = B * S                    # 4096 rows
M_CORE = M_FULL // MSH            # 1024 rows per core
N_CORE = F // NSH                 # 2048 cols per core
NT = 512                          # n-tile (one PSUM bank of f32)
WM, WK, WN = M_CORE // P, D // P, N_CORE // NT
XDMA = 4                          # xT load split (2 k-tiles per DMA)

_CACHE: dict = {}


def _build():
    import concourse.tile as tile
    from concourse import bacc, mybir

    nc = bacc.Bacc("TRN2", target_bir_lowering=False, debug=False)

    xt_dram = nc.dram_tensor("xt", [D, M_CORE], mybir.dt.float16, kind="ExternalInput")
    w_dram = nc.dram_tensor("w", [D, N_CORE], mybir.dt.int8, kind="ExternalInput")
    s_dram = nc.dram_tensor("s", [1, N_CORE], mybir.dt.float32, kind="ExternalInput")
    o_dram = nc.dram_tensor("o", [M_CORE, N_CORE], mybir.dt.float32, kind="ExternalOutput")

    xt_view = xt_dram[:, :].rearrange("(kt kp) m -> kp kt m", kp=P)  # [128, 8, 1024]

    with tile.TileContext(nc) as tc:
        with (
            tc.tile_pool(name="wi", bufs=1) as wip,
            tc.tile_pool(name="w", bufs=1) as wp,
            tc.tile_pool(name="qs", bufs=1) as qp,
            tc.tile_pool(name="xh", bufs=1) as xhp,
            tc.tile_pool(name="o", bufs=10) as op,
            tc.tile_pool(name="ps", bufs=8, space="PSUM") as pp,
        ):
            # Weight k-tile 0 first (gates the first dequant + matmul); its
            # first half alone gates the first matmul, so load it separately.
            w_i8 = [
                wip.tile([P, N_CORE], mybir.dt.int8, name=f"wi{kt}", tag=f"wi{kt}")
                for kt in range(WK)
            ]
            wh = N_CORE // 2
            for kt in (0, 1):
                nc.gpsimd.dma_start(
                    w_i8[kt][:, 0:wh], w_dram[kt * P:(kt + 1) * P, 0:wh]
                )
                nc.gpsimd.dma_start(
                    w_i8[kt][:, wh:N_CORE], w_dram[kt * P:(kt + 1) * P, wh:N_CORE]
                )

            # PE warm-up: ~3.6us of dummy matmuls on zeros while loads are in
            # flight, so the HAM clock-gate is released (1.2 -> 2.4 GHz)
            # before the first real matmul issues.
            warm = wp.tile([P, NT], mybir.dt.float16, name="warm", tag="warm")
            nc.vector.memset(warm[:], 0)
            warm_ps = pp.tile([P, NT], mybir.dt.float32, name="warm_ps", tag="ps")
            for _ in range(44):
                nc.tensor.matmul(warm_ps[:, 0:P], warm[:, 0:P], warm[:, 0:P])

            # xT shard [128kp, 8kt, 1024m] fp16 on the Sync ring.
            xh = xhp.tile([P, WK, M_CORE], mybir.dt.float16, name="xh", tag="xh")
            kper = WK // XDMA
            for i in range(XDMA):
                nc.sync.dma_start(
                    xh[:, i * kper:(i + 1) * kper, :],
                    xt_view[:, i * kper:(i + 1) * kper, :],
                )

            for kt in range(2, WK):
                nc.gpsimd.dma_start(w_i8[kt][:], w_dram[kt * P:(kt + 1) * P, :])

            # Dequant int8 -> fp16 on the vector engine, in k order; k-tile 0
            # in halves so the first matmuls' columns are ready earliest.
            w_sb = [
                wp.tile([P, N_CORE], mybir.dt.float16, name=f"w{kt}", tag=f"w{kt}")
                for kt in range(WK)
            ]
            for kt in (0, 1):
                nc.vector.tensor_copy(w_sb[kt][:, 0:wh], w_i8[kt][:, 0:wh])
                nc.vector.tensor_copy(w_sb[kt][:, wh:N_CORE], w_i8[kt][:, wh:N_CORE])
            cv = [nc.vector.tensor_copy(w_sb[kt][:], w_i8[kt][:]) for kt in range(2, WK)]

            # Scale broadcast (1 MB DRE replication): deferred so its bytes
            # don't starve the critical early loads; lands before first drain.
            qs = qp.tile([P, N_CORE], mybir.dt.float32)
            qs_dma = nc.scalar.dma_start(qs[:], s_dram[0:1, :].to_broadcast((P, N_CORE)))
            tile.add_dep_helper(qs_dma.ins, cv[0].ins, reason="defer qs broadcast")

            def drain(mi, nt, ps_tile):
                ot = op.tile([P, NT], mybir.dt.float32, name=f"o{mi}_{nt}", tag="o")
                nc.vector.tensor_mul(ot[:], ps_tile[:], qs[:, nt * NT:(nt + 1) * NT])
                nc.scalar.dma_start(
                    o_dram[mi * P:(mi + 1) * P, nt * NT:(nt + 1) * NT], ot[:]
                )

            def mm(ps_tile, kt, mi, nt, first, last):
                nc.tensor.matmul(
                    ps_tile[:],
                    xh[:, kt, mi * P:(mi + 1) * P],
                    w_sb[kt][:, nt * NT:(nt + 1) * NT],
                    start=first,
                    stop=last,
                )

            # m-tile pairs x 4 n-tiles = 8 PSUM banks per k-outer sweep.
            pairs = [(2 * i, 2 * i + 1) for i in range(WM // 2)]
            for pi, pair in enumerate(pairs):
                combos = [(mi, nt) for mi in pair for nt in range(WN)]
                if pi < len(pairs) - 1:
                    # k-outer: consume each weight k-tile across all 8 banks
                    # as soon as it is dequantized.
                    ps = {
                        c: pp.tile([P, NT], mybir.dt.float32, name=f"ps{pi}_{c[0]}_{c[1]}", tag="ps")
                        for c in combos
                    }
                    for kt in range(WK):
                        for c in combos:
                            mm(ps[c], kt, c[0], c[1], kt == 0, kt == WK - 1)
                    for c in combos:
                        drain(c[0], c[1], ps[c])
                else:
                    # Last sweep: n-outer so each bank's reduction finishes
                    # early and the tail drains overlap the remaining matmuls.
                    for c in combos:
                        ps_t = pp.tile([P, NT], mybir.dt.float32, name=f"ps{pi}_{c[0]}_{c[1]}", tag="ps")
                        for kt in range(WK):
                            mm(ps_t, kt, c[0], c[1], kt == 0, kt == WK - 1)
                        drain(c[0], c[1], ps_t)

    nc.compile()
    return nc


def _get_nc():
    if "nc" not in _CACHE:
        _CACHE["nc"] = _build()
    return _CACHE["nc"]


def _run(x, qkernel, qscale, trace=False):
    from concourse.bass_utils import run_bass_kernel_spmd

    x = np.asarray(x, dtype=np.float32).reshape(M_FULL, D)
    xt = np.ascontiguousarray(x.T).astype(np.float16)    # [D, M_FULL]
    w = np.asarray(qkernel)
    if w.dtype != np.int8:
        w = w.astype(np.int8)
    s = np.asarray(qscale, dtype=np.float32).reshape(1, F)

    in_maps = []
    for c in range(N_CORES):
        mb, nb = c % MSH, c // MSH
        in_maps.append({
            "xt": np.ascontiguousarray(xt[:, mb * M_CORE:(mb + 1) * M_CORE]),
            "w": np.ascontiguousarray(w[:, nb * N_CORE:(nb + 1) * N_CORE]),
            "s": np.ascontiguousarray(s[:, nb * N_CORE:(nb + 1) * N_CORE]),
        })
    res = run_bass_kernel_spmd(
        _get_nc(), in_maps, core_ids=list(range(N_CORES)), trace=trace
    )
    out = np.empty((M_FULL, F), dtype=np.float32)
    for c in range(N_CORES):
        mb, nb = c % MSH, c // MSH
        out[mb * M_CORE:(mb + 1) * M_CORE, nb * N_CORE:(nb + 1) * N_CORE] = res.results[c]["o"]
    return out.reshape(B, S, F), res


def kernel(x, qkernel, qscale):
    out, _ = _run(x, qkernel, qscale, trace=False)
    return out


def kernel_traced(x, qkernel, qscale):
    out, res = _run(x, qkernel, qscale, trace=True)
    return out, res
